# revision 64
# baseline (speedup 1.0000x reference)
"""GCN encoder (2x GCNConv + linear projection, relu) on 8 Trainium2 cores.

Self-contained: hardcodes the problem shapes (N=50000, E=800000, C=128,
OUT_C=64) and the sharding strategy.  Host side does structural prep only
(edge partitioning/sorting/padding, index-list construction, integer degree
counting); all FP math (matmuls, rsqrt, scaling, aggregation, bias, relu)
runs on device.

Math identity used on device (per GCNConv layer, with self-loops appended
to the edge list as ordinary edges):
    agg[d]  = sum_{e: dst=d} w_e * t[src_e]      w_e = rsqrt(deg_s)*rsqrt(deg_d)
    out[d]  = relu(W @ agg[d] + b)
i.e. the linear transform is applied AFTER aggregation and the edge
normalization is folded into the selection matrices:
    sel[e, j] = (rel_e == j) * w_e
built in batched DVE ops with the chunk dim innermost (dense inner strides
keep the DVE out of its slow broadcast path).  The aggregation matmul is
    aggT[f, d] += msg[e, f]^T(contraction) @ sel[e, d]
which leaves features on partitions, so the layer-2 epilogue can emit the
transposed hidden state (for the projection matmul) without any explicit
transposes.  Layer 2's table is pre-scaled by dinv (free, via the ACT
relu scale in the layer-1 epilogue), so its sel stays a plain one-hot and
the dst-side dinv is applied via a partition-replicated degree row.

Data movement per core:
  - layer 1 "gather": the host pre-permutes raw x rows into the exact
    (chunk, partition) consumption order (structural replication only),
    so layer 1 streams its messages with large sequential HWDGE DMAs and
    the Pool engine does zero descriptor generation for it
  - layer 2 gather: gpsimd dma_gather (bf16 256B rows) from the
    AllGathered table, pieces of ~4 windows alternating table halves
    (int16 index limit); Pool-engine descriptor generation (~3-6ns/row)
    is the dominant cost of the whole kernel
  - table row ids remapped to (ag_group, core, row) so the 4 pipelined
    partial AllGathers of the layer-1 output write contiguous row blocks
    and overlap the layer-1 compute
  - segment-sum: selection-matrix matmuls accumulating into a
    [128 feat x 128 dst] fp32 PSUM tile per window
"""

import sys
import numpy as np

for _p in ("/opt/trn_rl_repo",):
    if _p not in sys.path:
        sys.path.append(_p)

import concourse.bacc as bacc
import concourse.tile as tile
from concourse import bass, mybir, bass_utils

F32 = mybir.dt.float32
BF16 = mybir.dt.bfloat16
I16 = mybir.dt.int16
AF = mybir.ActivationFunctionType
ALU = mybir.AluOpType
NP_BF16 = mybir.dt.np(BF16)


class Cfg:
    def __init__(self, n_nodes, n_edges, cores=8, in_c=128, hid_c=128, out_c=64):
        assert in_c == 128 and hid_c == 128
        self.N, self.E, self.CORES = n_nodes, n_edges, cores
        self.C, self.OUT_C = in_c, out_c
        assert n_nodes % cores == 0
        self.S = n_nodes // cores                       # real nodes per shard
        self.SP = -(-self.S // 128) * 128               # padded shard rows
        self.NW = self.SP // 128                        # windows per core
        self.NPAD = self.SP * cores                     # padded table rows
        # AllGather pipeline groups (windows per group); must sum to NW and
        # the first half's table rows must stay under the int16 limit.
        self.AGW = [14, 12, 16, 7]
        assert sum(self.AGW) == self.NW
        self.AGS = np.cumsum([0] + self.AGW)[:-1].tolist()   # window starts
        self.AGR = [g * 128 for g in self.AGW]               # rows/core/group
        self.TB = np.cumsum([0] + [cores * r for r in self.AGR])[:-1].tolist()
        self.HALF = self.TB[len(self.AGW) // 2]         # int16 table split
        assert self.HALF < 32768 and self.NPAD - self.HALF < 32768
        self.GWIN = 3                                   # windows per gather piece
        self.SELB = 8                                  # chunks per sel batch


CFG = Cfg(50000, 800000)


def _wrap16(a):
    """[L] -> [128, L/16] int16 idx layout for dma_gather (16-wrap, 8x repl)."""
    assert a.size % 16 == 0
    w = a.reshape(-1, 16).T.astype(np.int16)
    return np.ascontiguousarray(np.tile(w, (8, 1)))


def _rid(cfg, n):
    """Remap node id -> table row id grouped by (ag_group, core, row)."""
    r = n // cfg.S
    l = n - r * cfg.S
    wl = l // 128
    p = l - wl * 128
    grp = np.digitize(wl, np.cumsum(cfg.AGW))           # 0..3
    ags = np.asarray(cfg.AGS)[grp]
    agr = np.asarray(cfg.AGR)[grp]
    tb = np.asarray(cfg.TB)[grp]
    return tb + r * agr + (wl - ags) * 128 + p


def _host_prep(cfg, x, edge_index):
    """Build per-core device inputs + the compile-time chunk schedule."""
    N, C, S, NW, CORES, HALF = cfg.N, cfg.C, cfg.S, cfg.NW, cfg.CORES, cfg.HALF

    src = np.asarray(edge_index[0]).astype(np.int64)
    dst = np.asarray(edge_index[1]).astype(np.int64)
    deg = (np.bincount(dst, minlength=N) + 1).astype(np.float32)

    # self-loops as ordinary edges
    loop = np.arange(N, dtype=np.int64)
    s_all = np.concatenate([src, loop])
    d_all = np.concatenate([dst, loop])

    srid = _rid(cfg, s_all)
    owner = d_all // S
    loc = d_all - owner * S
    win = loc // 128
    rel = (loc % 128).astype(np.float32)
    hB = (srid >= HALF).astype(np.int64)
    degs = deg[s_all]
    degd = deg[d_all]

    key = (owner * NW + win) * 2 + hB
    counts = np.bincount(key, minlength=CORES * NW * 2).reshape(CORES, NW, 2)
    caps = -(-counts.max(axis=0) // 128)                 # [NW, 2] chunks

    xbf = np.asarray(x, np.float32).astype(NP_BF16)

    per_core = []
    for c in range(CORES):
        m = owner == c
        cw, ch, cs, cr = win[m], hB[m], srid[m], rel[m]
        cds, cdd = degs[m], degd[m]
        order = np.lexsort((cs, ch, cw))
        cw, ch, cs, cr = cw[order], ch[order], cs[order], cr[order]
        cds, cdd = cds[order], cdd[order]
        k = cw * 2 + ch
        # idx streams: per half, windows in order, each (w,half) padded to
        # cap*128 (pad idx 0 -> fetches a real row, killed by rel=-1 sel)
        ia_parts, ib_parts = [], []
        for w in range(NW):
            for hf in (0, 1):
                cap = caps[w, hf]
                if cap == 0:
                    continue
                lo = np.searchsorted(k, w * 2 + hf, "left")
                hi = np.searchsorted(k, w * 2 + hf, "right")
                n = hi - lo
                assert n <= cap * 128
                iv = cs[lo:hi] - (HALF if hf else 0)
                iv = np.concatenate([iv, np.zeros(cap * 128 - n, np.int64)])
                (ib_parts if hf else ia_parts).append(iv)
        # rel/deg streams in chunk-consumption order:
        # for gather group: for half: for w in group: chunks
        # local-degree tiles for the dst-side normalization
        locnodes = np.arange(cfg.SP, dtype=np.int64)
        degloc = np.ones(cfg.SP, np.float32)
        real = locnodes < S
        degloc[real] = deg[c * S + locnodes[real]]
        degl = np.ascontiguousarray(degloc.reshape(NW, 128).T)     # [128, NW]
        degrow = np.ascontiguousarray(
            np.tile(degloc[None], (128, 1))).astype(NP_BF16)       # [128, SP]
        rel_parts, dgs_parts, dgd_parts, sid_parts = [], [], [], []
        csrc = s_all[m][order]                     # original src node ids
        for g0 in range(0, NW, cfg.GWIN):
            g1 = min(g0 + cfg.GWIN, NW)
            for hf in (0, 1):
                for w in range(g0, g1):
                    cap = caps[w, hf]
                    if cap == 0:
                        continue
                    lo = np.searchsorted(k, w * 2 + hf, "left")
                    hi = np.searchsorted(k, w * 2 + hf, "right")
                    n = hi - lo
                    pad = cap * 128 - n
                    rel_parts.append(np.concatenate(
                        [cr[lo:hi], np.full(pad, -1.0, np.float32)]))
                    dgs_parts.append(np.concatenate(
                        [cds[lo:hi], np.ones(pad, np.float32)]))
                    dgd_parts.append(np.concatenate(
                        [cdd[lo:hi], np.ones(pad, np.float32)]))
                    sid_parts.append(np.concatenate(
                        [csrc[lo:hi], np.zeros(pad, np.int64)]))
        idxa = np.concatenate(ia_parts) if ia_parts else np.zeros(16, np.int64)
        idxb = np.concatenate(ib_parts) if ib_parts else np.zeros(16, np.int64)

        def colT(parts):
            a = np.concatenate(parts).astype(np.float32)
            return np.ascontiguousarray(a.reshape(-1, 128).T).astype(NP_BF16)

        # layer-1 message stream: host pre-permutes raw x rows into the
        # exact (chunk, partition) consumption order (structural row
        # replication only — all scaling still happens on device via sel)
        sids = np.concatenate(sid_parts)
        xs = np.ascontiguousarray(
            xbf[sids].reshape(-1, 128, C).transpose(1, 0, 2))   # [128, nch, C]

        per_core.append(dict(
            idxa=_wrap16(idxa), idxb=_wrap16(idxb),
            rel=colT(rel_parts), degs=colT(dgs_parts), degd=colT(dgd_parts),
            degl=degl, degrow=degrow, xs=xs))

    sched = dict(caps=caps.tolist())
    return sched, {}, per_core


def _build_nc(cfg, sched):
    C, OUT_C = cfg.C, cfg.OUT_C
    SP, NPAD, HALF, NW, CORES = cfg.SP, cfg.NPAD, cfg.HALF, cfg.NW, cfg.CORES
    GWIN, SELB = cfg.GWIN, cfg.SELB
    AGW, AGS, AGR, TB = cfg.AGW, cfg.AGS, cfg.AGR, cfg.TB
    caps = sched["caps"]                                  # [NW][2]
    nchunk = int(sum(a + b for a, b in caps))
    la16 = sum(a for a, _ in caps) * 8                    # idx cols per half
    lb16 = sum(b for _, b in caps) * 8
    # gather pieces: (group, half) -> block count
    groups = [(g0, min(g0 + GWIN, NW)) for g0 in range(0, NW, GWIN)]
    pieceblk = [[sum(caps[w][hf] for w in range(g0, g1)) for hf in (0, 1)]
                for (g0, g1) in groups]
    maxblk = max(max(p) for p in pieceblk)

    nc = bacc.Bacc("TRN2", target_bir_lowering=False, debug=False,
                   enable_asserts=False, num_devices=CORES,
                   num_swdge_queues=4)

    def inp(name, shape, dt=F32):
        return nc.dram_tensor(name, shape, dt, kind="ExternalInput").ap()

    xs_d = inp("xs", [128, nchunk, C], BF16)
    w1t_d = inp("w1t", [C, C], BF16)
    w2t_d = inp("w2t", [C, C], BF16)
    wpt_d = inp("wpt", [C, OUT_C], BF16)
    b1b_d = inp("b1b", [128, C])
    b2c_d = inp("b2c", [128, 1])
    bpb_d = inp("bpb", [128, OUT_C])
    # iota with the chunk dim innermost: iotaX[p, j*SELB + k] = j, so the
    # broadcast operands in the sel build keep a dense inner stride (DVE 2x
    # perf mode needs step_x=1 on every operand)
    iota_d = inp("iota", [128, 128 * SELB], BF16)
    idxa_d = inp("idxa", [128, max(la16, 16)], I16)
    idxb_d = inp("idxb", [128, max(lb16, 16)], I16)
    rel_d = inp("rel", [128, nchunk], BF16)
    degs_d = inp("degs", [128, nchunk], BF16)
    degd_d = inp("degd", [128, nchunk], BF16)
    degl_d = inp("degl", [128, NW])
    degrow_d = inp("degrow", [128, SP], BF16)
    out_d = nc.dram_tensor("out", [SP, OUT_C], F32, kind="ExternalOutput").ap()

    g2loc = [nc.dram_tensor(f"g2loc{g}", [AGR[g], C], BF16, kind="Internal").ap()
             for g in range(len(AGW))]
    g2d = nc.dram_tensor("g2d", [NPAD, C], BF16, kind="Internal",
                         addr_space="Shared").ap()

    from contextlib import ExitStack
    with tile.TileContext(nc) as tc, ExitStack() as ctx:
        cp = ctx.enter_context(tc.tile_pool(name="consts", bufs=1))
        msgp = ctx.enter_context(tc.tile_pool(name="msg", bufs=5))
        selp = ctx.enter_context(tc.tile_pool(name="sel", bufs=8))
        epool = ctx.enter_context(tc.tile_pool(name="epi", bufs=6))
        sqp = ctx.enter_context(tc.tile_pool(name="sqtmp", bufs=1))
        stgp = ctx.enter_context(tc.tile_pool(name="stg", bufs=1))
        ppool_w = ctx.enter_context(tc.tile_pool(name="psw", bufs=5, space="PSUM"))
        ppool_e = ctx.enter_context(tc.tile_pool(name="pse", bufs=2, space="PSUM"))
        ppool_y = ctx.enter_context(tc.tile_pool(name="psy", bufs=1, space="PSUM"))

        def cload(name, ap, shape, dt=F32):
            t = cp.tile(shape, dt, tag=name)
            nc.sync.dma_start(t[:], ap[:])
            return t

        w1t = cload("w1t", w1t_d, [C, C], BF16)
        w2t = cload("w2t", w2t_d, [C, C], BF16)
        wpt = cload("wpt", wpt_d, [C, OUT_C], BF16)
        b1b = cload("b1b", b1b_d, [128, C])
        b2c = cload("b2c", b2c_d, [128, 1])
        bpb = cload("bpb", bpb_d, [128, OUT_C])
        iota = cload("iota", iota_d, [128, 128 * SELB], BF16)
        idxa = cload("idxa", idxa_d, [128, max(la16, 16)], I16)
        idxb = cload("idxb", idxb_d, [128, max(lb16, 16)], I16)
        rel = cload("rel", rel_d, [128, nchunk], BF16)
        degs = cload("degs", degs_d, [128, nchunk], BF16)
        degd = cload("degd", degd_d, [128, nchunk], BF16)
        degl = cload("degl", degl_d, [128, NW])
        degrow = cload("degrow", degrow_d, [128, SP], BF16)

        # per-edge norm w = rsqrt(deg_s) * rsqrt(deg_d), on device (layer 1)
        t_s = cp.tile([128, nchunk], F32, tag="t_s")
        nc.scalar.activation(t_s[:], degs[:], AF.Sqrt)
        r_s = cp.tile([128, nchunk], F32, tag="r_s")
        nc.vector.reciprocal(r_s[:], t_s[:])
        t_d = cp.tile([128, nchunk], F32, tag="t_d")
        nc.scalar.activation(t_d[:], degd[:], AF.Sqrt)
        r_d = cp.tile([128, nchunk], F32, tag="r_d")
        nc.vector.reciprocal(r_d[:], t_d[:])
        ws = cp.tile([128, nchunk], BF16, tag="ws")
        nc.vector.tensor_tensor(out=ws[:], in0=r_s[:], in1=r_d[:], op=ALU.mult)
        # dst-side rsqrt(deg): per-window column [128, NW] and a
        # partition-replicated row [128, SP] for the layer-2 epilogue
        sql = cp.tile([128, NW], F32, tag="sql")
        nc.scalar.activation(sql[:], degl[:], AF.Sqrt)
        dinvl = cp.tile([128, NW], F32, tag="dinvl")
        nc.vector.reciprocal(dinvl[:], sql[:])
        grp_of = []
        for g in range(len(AGW)):
            grp_of += [g] * AGW[g]

        def layer(tabA, tabB, is_l1, h1st):
            ci = 0           # chunk cursor (rel/ws stream)
            offa = offb = 0  # idx col cursors
            selts = {}       # batch index -> sel tile

            def sel_build(cb):
                # sel layout [128 e, 128 d, SELB chunk]: chunk innermost so
                # every DVE operand streams with step 1 (iota is materialized
                # in this layout; rel/ws broadcast over the d dim only).
                # The matmul rhs slice is strided by SELB, which costs some
                # TensorE rhs bandwidth but keeps the DVE build ~6x faster.
                nb = min(SELB, nchunk - cb)
                st = selp.tile([128, 128, SELB], BF16, tag="sel", name="sel")
                iav = iota[:].rearrange("p (j k) -> p j k", k=SELB)
                nc.vector.tensor_tensor(
                    out=st[:, :, :nb] if nb < SELB else st[:],
                    in0=iav[:, :, :nb] if nb < SELB else iav,
                    in1=rel[:, None, cb:cb + nb].to_broadcast([128, 128, nb]),
                    op=ALU.is_equal)
                if is_l1:
                    # layer 1 gathers raw x: per-edge dinv_s*dinv_d in sel.
                    # layer 2's table is pre-scaled by dinv, dst side is
                    # applied in the epilogue, so its sel stays one-hot.
                    nc.vector.tensor_tensor(
                        out=st[:, :, :nb] if nb < SELB else st[:],
                        in0=st[:, :, :nb] if nb < SELB else st[:],
                        in1=ws[:, None, cb:cb + nb].to_broadcast([128, 128, nb]),
                        op=ALU.mult)
                selts[cb // SELB] = st

            def sel_slice(ci):
                return selts[ci // SELB][:, :, ci % SELB]

            # precompute per-piece chunk starts and idx column offsets
            pc0, poff = {}, {}
            cc = 0
            oa = ob = 0
            for gi in range(len(groups)):
                for hf in (0, 1):
                    b = pieceblk[gi][hf]
                    pc0[(gi, hf)] = cc
                    poff[(gi, hf)] = oa if hf == 0 else ob
                    cc += b
                    if hf == 0:
                        oa += b * 8
                    else:
                        ob += b * 8

            msgs = {}

            def emit_gather(gi, hf):
                blocks = pieceblk[gi][hf]
                if blocks == 0:
                    return
                c0 = pc0[(gi, hf)]
                nidx = blocks * 128
                msg = msgp.tile([128, maxblk, C], BF16, tag=f"msg{hf}",
                                name=f"m{gi}_{hf}")
                if is_l1:
                    # layer 1: host-prepermuted stream, plain big DMA
                    nc.sync.dma_start(msg[:, :blocks, :],
                                      xs_d[:, c0:c0 + blocks, :])
                elif hf == 0:
                    isl = idxa[:, poff[(gi, hf)]:poff[(gi, hf)] + nidx // 16]
                    nc.gpsimd.dma_gather(msg[:, :blocks, :], tabA, isl,
                                         nidx, nidx, elem_size=C,
                                         single_packet=False,
                                         queue_num=(gi * 2 + hf) % 4)
                else:
                    isl = idxb[:, poff[(gi, hf)]:poff[(gi, hf)] + nidx // 16]
                    nc.gpsimd.dma_gather(msg[:, :blocks, :], tabB, isl,
                                         nidx, nidx, elem_size=C,
                                         single_packet=False,
                                         queue_num=(gi * 2 + hf) % 4)
                msgs[(gi, hf)] = msg

            def epilogues(g0, g1, ps):
                for w in range(g0, g1):
                    aT = epool.tile([128, 128], BF16, tag="aT")
                    nc.scalar.activation(aT[:], ps[w][:], AF.Identity)
                    if is_l1:
                        pre = ppool_e.tile([128, C], F32, tag="pse")
                        nc.tensor.matmul(pre[:], lhsT=aT[:], rhs=w1t[:],
                                         start=True, stop=True)
                        t1 = epool.tile([128, C], F32, tag="t1")
                        nc.vector.tensor_tensor(out=t1[:], in0=pre[:],
                                                in1=b1b[:], op=ALU.add)
                        # table2 row = dinv_d * relu(t1) = relu(dinv_d * t1)
                        g = grp_of[w]
                        nc.scalar.activation(h1st[g][:, w - AGS[g], :], t1[:],
                                             AF.Relu, scale=dinvl[:, w:w + 1])
                        if w == AGS[g] + AGW[g] - 1:
                            nc.sync.dma_start(
                                g2loc[g][:].rearrange("(j p) f -> p j f", p=128),
                                h1st[g][:])
                            nc.gpsimd.collective_compute(
                                "AllGather", ALU.bypass,
                                replica_groups=[list(range(CORES))],
                                ins=[g2loc[g][:]],
                                outs=[g2d[TB[g]:TB[g] + CORES * AGR[g], :]])
                    else:
                        p2 = ppool_e.tile([128, 128], F32, tag="pse")
                        nc.tensor.matmul(p2[:], lhsT=w2t[:], rhs=aT[:],
                                         start=True, stop=True)
                        # dst-side dinv_d (d on the free dim here)
                        t2 = epool.tile([128, 128], F32, tag="t2")
                        nc.vector.tensor_tensor(
                            out=t2[:], in0=p2[:],
                            in1=dinvb[:, w * 128:(w + 1) * 128], op=ALU.mult)
                        h2T = epool.tile([128, 128], BF16, tag="h2T")
                        nc.scalar.activation(h2T[:], t2[:], AF.Relu,
                                             bias=b2c[:, 0:1])
                        yp = ppool_y.tile([128, OUT_C], F32, tag="psy")
                        nc.tensor.matmul(yp[:], lhsT=h2T[:], rhs=wpt[:],
                                         start=True, stop=True)
                        yt = epool.tile([128, OUT_C], F32, tag="yt")
                        nc.vector.tensor_tensor(out=yt[:], in0=yp[:],
                                                in1=bpb[:], op=ALU.add)
                        yr = epool.tile([128, OUT_C], F32, tag="yr")
                        nc.scalar.activation(yr[:], yt[:], AF.Relu)
                        nc.sync.dma_start(out_d[w * 128:(w + 1) * 128, :], yr[:])


            # emit each group's gather pieces just ahead of their matmuls
            for p0 in range(0, len(groups), 1):
                pair = [p0]
                for gj in pair:
                    emit_gather(gj, 0)
                    emit_gather(gj, 1)
                for gj in pair:
                    g0, g1 = groups[gj]
                    gchunks = sum(caps[w][0] + caps[w][1]
                                  for w in range(g0, g1))
                    for cb in range((ci // SELB) * SELB, ci + gchunks, SELB):
                        if cb // SELB not in selts and cb < nchunk:
                            sel_build(cb)
                    # aggregation: aggT[f, d] += msg^T-contraction @ sel
                    done = {w: 0 for w in range(g0, g1)}
                    tot = {w: caps[w][0] + caps[w][1] for w in range(g0, g1)}
                    ps = {}
                    for hf in (0, 1):
                        blk = 0
                        for w in range(g0, g1):
                            for _ in range(caps[w][hf]):
                                sl = sel_slice(ci)
                                ci += 1
                                if w not in ps:
                                    ps[w] = ppool_w.tile([128, 128], F32,
                                                         tag="psw",
                                                         name=f"ps{w}")
                                nc.tensor.matmul(
                                    ps[w][:], lhsT=msgs[(gj, hf)][:, blk, :],
                                    rhs=sl, start=(done[w] == 0),
                                    stop=(done[w] == tot[w] - 1))
                                done[w] += 1
                                blk += 1
                    epilogues(g0, g1, ps)

        h1st = [stgp.tile([128, AGW[g], C], BF16, tag=f"h1st{g}",
                          name=f"h1st{g}") for g in range(len(AGW))]
        layer(None, None, True, h1st)
        # dinvb is only read by the layer-2 epilogue; emitting its
        # DVE reciprocals here keeps them off the startup critical path
        dinvb = cp.tile([128, SP], BF16, tag="dinvb")
        for j in range(0, SP, 1568):
            sqb = sqp.tile([128, 1568], F32, tag="sqb")
            nc.scalar.activation(sqb[:], degrow[:, j:j + 1568], AF.Sqrt)
            with nc.allow_low_precision(reason="bf16 dinv, |err|<0.4% ok"):
                nc.vector.reciprocal(dinvb[:, j:j + 1568], sqb[:])

        layer(g2d[0:HALF, :], g2d[HALF:NPAD, :], False, None)

    nc.compile()
    return nc


def _make_in_maps(cfg, shared, per_core, W1, b1, W2, b2, Wp, bp):
    w1t = np.ascontiguousarray(np.asarray(W1, np.float32).T).astype(NP_BF16)
    w2t = np.ascontiguousarray(np.asarray(W2, np.float32).T).astype(NP_BF16)
    wpt = np.ascontiguousarray(np.asarray(Wp, np.float32).T).astype(NP_BF16)
    b1b = np.ascontiguousarray(np.tile(np.asarray(b1, np.float32)[None], (128, 1)))
    b2c = np.ascontiguousarray(np.asarray(b2, np.float32)[:, None])
    bpb = np.ascontiguousarray(np.tile(np.asarray(bp, np.float32)[None], (128, 1)))
    iota = np.ascontiguousarray(np.tile(
        np.repeat(np.arange(128, dtype=np.float32), cfg.SELB)[None],
        (128, 1))).astype(NP_BF16)
    base = dict(w1t=w1t, w2t=w2t, wpt=wpt, b1b=b1b, b2c=b2c,
                bpb=bpb, iota=iota)
    in_maps = []
    for c in range(cfg.CORES):
        pc = per_core[c]
        m = dict(base)
        m["idxa"] = pc["idxa"]
        m["idxb"] = pc["idxb"]
        m["rel"] = pc["rel"]
        m["degs"] = pc["degs"]
        m["degd"] = pc["degd"]
        m["degl"] = pc["degl"]
        m["degrow"] = pc["degrow"]
        m["xs"] = pc["xs"]
        in_maps.append(m)
    return in_maps


def _run(inputs, cfg=None, trace=False, tmpdir=None, verbose=True):
    import time
    t0 = time.time()

    def _log(msg):
        if verbose:
            print(f"[kernel {time.time()-t0:7.1f}s] {msg}", flush=True)
    cfg = cfg or CFG
    sched, shared, per_core = _host_prep(cfg, inputs["x"], inputs["edge_index"])
    _log("host prep done")
    nc = _build_nc(cfg, sched)
    _log("build+compile done")
    in_maps = _make_in_maps(cfg, shared, per_core,
                            inputs["W1"], inputs["b1"], inputs["W2"],
                            inputs["b2"], inputs["Wp"], inputs["bp"])
    _log("in_maps done")
    core_ids = list(range(cfg.CORES))
    if trace:
        # NTFF profiling needs a warm first execute; run once untraced.
        bass_utils.run_bass_kernel_spmd(nc, in_maps, core_ids=core_ids,
                                        trace=False)
        _log("warmup run done")
    res = bass_utils.run_bass_kernel_spmd(nc, in_maps, core_ids=core_ids,
                                          trace=trace, tmpdir=tmpdir)
    _log("run done")
    out = np.empty((cfg.N, cfg.OUT_C), np.float32)
    for c in range(cfg.CORES):
        out[c * cfg.S:(c + 1) * cfg.S] = res.results[c]["out"][:cfg.S]
    return out, res


def kernel(**inputs):
    out, _ = _run(inputs)
    return out


# revision 65
# speedup vs baseline: 1.0674x; 1.0674x over previous
"""GCN encoder (2x GCNConv + linear projection, relu) on 8 Trainium2 cores.

Self-contained: hardcodes the problem shapes (N=50000, E=800000, C=128,
OUT_C=64) and the sharding strategy.  Host side does structural prep only
(edge partitioning/sorting/padding, index-list construction, integer degree
counting); all FP math (matmuls, rsqrt, scaling, aggregation, bias, relu)
runs on device.

Math identity used on device (per GCNConv layer, with self-loops appended
to the edge list as ordinary edges):
    agg[d]  = sum_{e: dst=d} w_e * t[src_e]      w_e = rsqrt(deg_s)*rsqrt(deg_d)
    out[d]  = relu(W @ agg[d] + b)
i.e. the linear transform is applied AFTER aggregation and the edge
normalization is folded into the selection matrices:
    sel[e, j] = (rel_e == j) * w_e
built in batched DVE ops with the chunk dim innermost (dense inner strides
keep the DVE out of its slow broadcast path).  The aggregation matmul is
    aggT[f, d] += msg[e, f]^T(contraction) @ sel[e, d]
which leaves features on partitions, so the layer-2 epilogue can emit the
transposed hidden state (for the projection matmul) without any explicit
transposes.  Layer 2's table is pre-scaled by dinv (free, via the ACT
relu scale in the layer-1 epilogue), so its sel stays a plain one-hot and
the dst-side dinv is applied via a partition-replicated degree row.

Data movement per core:
  - layer 1 "gather": the host pre-permutes raw x rows into the exact
    (chunk, partition) consumption order (structural replication only),
    so layer 1 streams its messages with large sequential HWDGE DMAs and
    the Pool engine does zero descriptor generation for it
  - layer 2 gather: gpsimd dma_gather (bf16 256B rows) from the
    AllGathered table, pieces of ~4 windows alternating table halves
    (int16 index limit); Pool-engine descriptor generation (~3-6ns/row)
    is the dominant cost of the whole kernel
  - table row ids remapped to (ag_group, core, row) so the 4 pipelined
    partial AllGathers of the layer-1 output write contiguous row blocks
    and overlap the layer-1 compute
  - segment-sum: selection-matrix matmuls accumulating into a
    [128 feat x 128 dst] fp32 PSUM tile per window
"""

import sys
import numpy as np

for _p in ("/opt/trn_rl_repo",):
    if _p not in sys.path:
        sys.path.append(_p)

import concourse.bacc as bacc
import concourse.tile as tile
from concourse import bass, mybir, bass_utils

F32 = mybir.dt.float32
BF16 = mybir.dt.bfloat16
I16 = mybir.dt.int16
AF = mybir.ActivationFunctionType
ALU = mybir.AluOpType
NP_BF16 = mybir.dt.np(BF16)


class Cfg:
    def __init__(self, n_nodes, n_edges, cores=8, in_c=128, hid_c=128, out_c=64):
        assert in_c == 128 and hid_c == 128
        self.N, self.E, self.CORES = n_nodes, n_edges, cores
        self.C, self.OUT_C = in_c, out_c
        assert n_nodes % cores == 0
        self.S = n_nodes // cores                       # real nodes per shard
        self.SP = -(-self.S // 128) * 128               # padded shard rows
        self.NW = self.SP // 128                        # windows per core
        self.NPAD = self.SP * cores                     # padded table rows
        # AllGather pipeline groups (windows per group); must sum to NW and
        # the first half's table rows must stay under the int16 limit.
        self.AGW = [14, 12, 16, 7]
        assert sum(self.AGW) == self.NW
        self.AGS = np.cumsum([0] + self.AGW)[:-1].tolist()   # window starts
        self.AGR = [g * 128 for g in self.AGW]               # rows/core/group
        self.TB = np.cumsum([0] + [cores * r for r in self.AGR])[:-1].tolist()
        self.HALF = self.TB[len(self.AGW) // 2]         # int16 table split
        assert self.HALF < 32768 and self.NPAD - self.HALF < 32768
        self.GWIN = 3                                   # windows per gather piece
        self.SELB = 8                                  # chunks per sel batch


CFG = Cfg(50000, 800000)


def _wrap16(a):
    """[L] -> [128, L/16] int16 idx layout for dma_gather (16-wrap, 8x repl)."""
    assert a.size % 16 == 0
    w = a.reshape(-1, 16).T.astype(np.int16)
    return np.ascontiguousarray(np.tile(w, (8, 1)))


def _rid(cfg, n):
    """Remap node id -> table row id grouped by (ag_group, core, row)."""
    r = n // cfg.S
    l = n - r * cfg.S
    wl = l // 128
    p = l - wl * 128
    grp = np.digitize(wl, np.cumsum(cfg.AGW))           # 0..3
    ags = np.asarray(cfg.AGS)[grp]
    agr = np.asarray(cfg.AGR)[grp]
    tb = np.asarray(cfg.TB)[grp]
    return tb + r * agr + (wl - ags) * 128 + p


def _host_prep(cfg, x, edge_index):
    """Build per-core device inputs + the compile-time chunk schedule."""
    N, C, S, NW, CORES, HALF = cfg.N, cfg.C, cfg.S, cfg.NW, cfg.CORES, cfg.HALF

    src = np.asarray(edge_index[0]).astype(np.int64)
    dst = np.asarray(edge_index[1]).astype(np.int64)
    deg = (np.bincount(dst, minlength=N) + 1).astype(np.float32)

    # self-loops as ordinary edges
    loop = np.arange(N, dtype=np.int64)
    s_all = np.concatenate([src, loop])
    d_all = np.concatenate([dst, loop])

    srid = _rid(cfg, s_all)
    owner = d_all // S
    loc = d_all - owner * S
    win = loc // 128
    rel = (loc % 128).astype(np.float32)
    hB = (srid >= HALF).astype(np.int64)
    degs = deg[s_all]
    degd = deg[d_all]

    key = (owner * NW + win) * 2 + hB
    counts = np.bincount(key, minlength=CORES * NW * 2).reshape(CORES, NW, 2)
    caps = -(-counts.max(axis=0) // 128)                 # [NW, 2] chunks

    xbf = np.asarray(x, np.float32).astype(NP_BF16)

    per_core = []
    for c in range(CORES):
        m = owner == c
        cw, ch, cs, cr = win[m], hB[m], srid[m], rel[m]
        cds, cdd = degs[m], degd[m]
        order = np.lexsort((cs, ch, cw))
        cw, ch, cs, cr = cw[order], ch[order], cs[order], cr[order]
        cds, cdd = cds[order], cdd[order]
        k = cw * 2 + ch
        # idx streams: per half, windows in order, each (w,half) padded to
        # cap*128 (pad idx 0 -> fetches a real row, killed by rel=-1 sel)
        ia_parts, ib_parts = [], []
        for w in range(NW):
            for hf in (0, 1):
                cap = caps[w, hf]
                if cap == 0:
                    continue
                lo = np.searchsorted(k, w * 2 + hf, "left")
                hi = np.searchsorted(k, w * 2 + hf, "right")
                n = hi - lo
                assert n <= cap * 128
                iv = cs[lo:hi] - (HALF if hf else 0)
                iv = np.concatenate([iv, np.zeros(cap * 128 - n, np.int64)])
                (ib_parts if hf else ia_parts).append(iv)
        # rel/deg streams in chunk-consumption order:
        # for gather group: for half: for w in group: chunks
        # local-degree tiles for the dst-side normalization
        locnodes = np.arange(cfg.SP, dtype=np.int64)
        degloc = np.ones(cfg.SP, np.float32)
        real = locnodes < S
        degloc[real] = deg[c * S + locnodes[real]]
        degl = np.ascontiguousarray(degloc.reshape(NW, 128).T)     # [128, NW]
        degrow = np.ascontiguousarray(
            np.tile(degloc[None], (128, 1))).astype(NP_BF16)       # [128, SP]
        rel_parts, dgs_parts, dgd_parts, sid_parts = [], [], [], []
        csrc = s_all[m][order]                     # original src node ids
        for g0 in range(0, NW, cfg.GWIN):
            g1 = min(g0 + cfg.GWIN, NW)
            for hf in (0, 1):
                for w in range(g0, g1):
                    cap = caps[w, hf]
                    if cap == 0:
                        continue
                    lo = np.searchsorted(k, w * 2 + hf, "left")
                    hi = np.searchsorted(k, w * 2 + hf, "right")
                    n = hi - lo
                    pad = cap * 128 - n
                    rel_parts.append(np.concatenate(
                        [cr[lo:hi], np.full(pad, -1.0, np.float32)]))
                    dgs_parts.append(np.concatenate(
                        [cds[lo:hi], np.ones(pad, np.float32)]))
                    dgd_parts.append(np.concatenate(
                        [cdd[lo:hi], np.ones(pad, np.float32)]))
                    sid_parts.append(np.concatenate(
                        [csrc[lo:hi], np.zeros(pad, np.int64)]))
        idxa = np.concatenate(ia_parts) if ia_parts else np.zeros(16, np.int64)
        idxb = np.concatenate(ib_parts) if ib_parts else np.zeros(16, np.int64)

        def colT(parts):
            a = np.concatenate(parts).astype(np.float32)
            return np.ascontiguousarray(a.reshape(-1, 128).T).astype(NP_BF16)

        # layer-1 message stream: host pre-permutes raw x rows into the
        # exact (chunk, partition) consumption order (structural row
        # replication only — all scaling still happens on device via sel)
        sids = np.concatenate(sid_parts)
        xs = np.ascontiguousarray(
            xbf[sids].reshape(-1, 128, C).transpose(1, 0, 2))   # [128, nch, C]

        per_core.append(dict(
            idxa=_wrap16(idxa), idxb=_wrap16(idxb),
            rel=colT(rel_parts), degs=colT(dgs_parts), degd=colT(dgd_parts),
            degl=degl, degrow=degrow, xs=xs))

    sched = dict(caps=caps.tolist())
    return sched, {}, per_core


def _build_nc(cfg, sched):
    C, OUT_C = cfg.C, cfg.OUT_C
    SP, NPAD, HALF, NW, CORES = cfg.SP, cfg.NPAD, cfg.HALF, cfg.NW, cfg.CORES
    GWIN, SELB = cfg.GWIN, cfg.SELB
    AGW, AGS, AGR, TB = cfg.AGW, cfg.AGS, cfg.AGR, cfg.TB
    caps = sched["caps"]                                  # [NW][2]
    nchunk = int(sum(a + b for a, b in caps))
    la16 = sum(a for a, _ in caps) * 8                    # idx cols per half
    lb16 = sum(b for _, b in caps) * 8
    # gather pieces: (group, half) -> block count
    groups = [(g0, min(g0 + GWIN, NW)) for g0 in range(0, NW, GWIN)]
    pieceblk = [[sum(caps[w][hf] for w in range(g0, g1)) for hf in (0, 1)]
                for (g0, g1) in groups]
    maxblk = max(max(p) for p in pieceblk)

    nc = bacc.Bacc("TRN2", target_bir_lowering=False, debug=False,
                   enable_asserts=False, num_devices=CORES,
                   num_swdge_queues=4)

    def inp(name, shape, dt=F32):
        return nc.dram_tensor(name, shape, dt, kind="ExternalInput").ap()

    xs_d = inp("xs", [128, nchunk, C], BF16)
    w1t_d = inp("w1t", [C, C], BF16)
    w2t_d = inp("w2t", [C, C], BF16)
    wpt_d = inp("wpt", [C, OUT_C], BF16)
    b1b_d = inp("b1b", [128, C])
    b2c_d = inp("b2c", [128, 1])
    bpb_d = inp("bpb", [128, OUT_C])
    # iota with the chunk dim innermost: iotaX[p, j*SELB + k] = j, so the
    # broadcast operands in the sel build keep a dense inner stride (DVE 2x
    # perf mode needs step_x=1 on every operand)
    iota_d = inp("iota", [128, 128 * SELB], BF16)
    idxa_d = inp("idxa", [128, max(la16, 16)], I16)
    idxb_d = inp("idxb", [128, max(lb16, 16)], I16)
    rel_d = inp("rel", [128, nchunk], BF16)
    degs_d = inp("degs", [128, nchunk], BF16)
    degd_d = inp("degd", [128, nchunk], BF16)
    degl_d = inp("degl", [128, NW])
    degrow_d = inp("degrow", [128, SP], BF16)
    out_d = nc.dram_tensor("out", [SP, OUT_C], F32, kind="ExternalOutput").ap()

    g2loc = [nc.dram_tensor(f"g2loc{g}", [AGR[g], C], BF16, kind="Internal").ap()
             for g in range(len(AGW))]
    g2d = nc.dram_tensor("g2d", [NPAD, C], BF16, kind="Internal",
                         addr_space="Shared").ap()

    from contextlib import ExitStack
    with tile.TileContext(nc) as tc, ExitStack() as ctx:
        cp = ctx.enter_context(tc.tile_pool(name="consts", bufs=1))
        msgp = ctx.enter_context(tc.tile_pool(name="msg", bufs=4))
        selp = ctx.enter_context(tc.tile_pool(name="sel", bufs=8))
        epool = ctx.enter_context(tc.tile_pool(name="epi", bufs=6))
        stgp = ctx.enter_context(tc.tile_pool(name="stg", bufs=1))
        ppool_w = ctx.enter_context(tc.tile_pool(name="psw", bufs=5, space="PSUM"))
        ppool_e = ctx.enter_context(tc.tile_pool(name="pse", bufs=2, space="PSUM"))
        ppool_y = ctx.enter_context(tc.tile_pool(name="psy", bufs=1, space="PSUM"))

        def cload(name, ap, shape, dt=F32):
            t = cp.tile(shape, dt, tag=name)
            nc.sync.dma_start(t[:], ap[:])
            return t

        w1t = cload("w1t", w1t_d, [C, C], BF16)
        w2t = cload("w2t", w2t_d, [C, C], BF16)
        wpt = cload("wpt", wpt_d, [C, OUT_C], BF16)
        b1b = cload("b1b", b1b_d, [128, C])
        b2c = cload("b2c", b2c_d, [128, 1])
        bpb = cload("bpb", bpb_d, [128, OUT_C])
        iota = cload("iota", iota_d, [128, 128 * SELB], BF16)
        idxa = cload("idxa", idxa_d, [128, max(la16, 16)], I16)
        idxb = cload("idxb", idxb_d, [128, max(lb16, 16)], I16)
        rel = cload("rel", rel_d, [128, nchunk], BF16)
        degs = cload("degs", degs_d, [128, nchunk], BF16)
        degd = cload("degd", degd_d, [128, nchunk], BF16)
        degl = cload("degl", degl_d, [128, NW])
        degrow = cload("degrow", degrow_d, [128, SP], BF16)

        # per-edge norm w = rsqrt(deg_s) * rsqrt(deg_d), on device (layer 1)
        t_s = cp.tile([128, nchunk], F32, tag="t_s")
        nc.scalar.activation(t_s[:], degs[:], AF.Sqrt)
        r_s = cp.tile([128, nchunk], F32, tag="r_s")
        nc.vector.reciprocal(r_s[:], t_s[:])
        t_d = cp.tile([128, nchunk], F32, tag="t_d")
        nc.scalar.activation(t_d[:], degd[:], AF.Sqrt)
        r_d = cp.tile([128, nchunk], F32, tag="r_d")
        nc.vector.reciprocal(r_d[:], t_d[:])
        ws = cp.tile([128, nchunk], BF16, tag="ws")
        nc.vector.tensor_tensor(out=ws[:], in0=r_s[:], in1=r_d[:], op=ALU.mult)
        # dst-side rsqrt(deg): per-window column [128, NW] and a
        # partition-replicated row [128, SP] for the layer-2 epilogue
        sql = cp.tile([128, NW], F32, tag="sql")
        nc.scalar.activation(sql[:], degl[:], AF.Sqrt)
        dinvl = cp.tile([128, NW], F32, tag="dinvl")
        nc.vector.reciprocal(dinvl[:], sql[:])
        grp_of = []
        for g in range(len(AGW)):
            grp_of += [g] * AGW[g]

        def layer(tabA, tabB, is_l1, h1st):
            ci = 0           # chunk cursor (rel/ws stream)
            offa = offb = 0  # idx col cursors
            selts = {}       # batch index -> sel tile

            def sel_build(cb):
                # sel layout [128 e, 128 d, SELB chunk]: chunk innermost so
                # every DVE operand streams with step 1 (iota is materialized
                # in this layout; rel/ws broadcast over the d dim only).
                # The matmul rhs slice is strided by SELB, which costs some
                # TensorE rhs bandwidth but keeps the DVE build ~6x faster.
                nb = min(SELB, nchunk - cb)
                st = selp.tile([128, 128, SELB], BF16, tag="sel", name="sel")
                iav = iota[:].rearrange("p (j k) -> p j k", k=SELB)
                nc.vector.tensor_tensor(
                    out=st[:, :, :nb] if nb < SELB else st[:],
                    in0=iav[:, :, :nb] if nb < SELB else iav,
                    in1=rel[:, None, cb:cb + nb].to_broadcast([128, 128, nb]),
                    op=ALU.is_equal)
                if is_l1:
                    # layer 1 gathers raw x: per-edge dinv_s*dinv_d in sel.
                    # layer 2's table is pre-scaled by dinv, dst side is
                    # applied in the epilogue, so its sel stays one-hot.
                    nc.vector.tensor_tensor(
                        out=st[:, :, :nb] if nb < SELB else st[:],
                        in0=st[:, :, :nb] if nb < SELB else st[:],
                        in1=ws[:, None, cb:cb + nb].to_broadcast([128, 128, nb]),
                        op=ALU.mult)
                selts[cb // SELB] = st

            def sel_slice(ci):
                return selts[ci // SELB][:, :, ci % SELB]

            # precompute per-piece chunk starts and idx column offsets
            pc0, poff = {}, {}
            cc = 0
            oa = ob = 0
            for gi in range(len(groups)):
                for hf in (0, 1):
                    b = pieceblk[gi][hf]
                    pc0[(gi, hf)] = cc
                    poff[(gi, hf)] = oa if hf == 0 else ob
                    cc += b
                    if hf == 0:
                        oa += b * 8
                    else:
                        ob += b * 8

            msgs = {}

            def emit_gather(gi, hf):
                blocks = pieceblk[gi][hf]
                if blocks == 0:
                    return
                c0 = pc0[(gi, hf)]
                nidx = blocks * 128
                msg = msgp.tile([128, maxblk, C], BF16, tag=f"msg{hf}",
                                name=f"m{gi}_{hf}")
                if is_l1:
                    # layer 1: host-prepermuted stream, plain big DMA
                    nc.sync.dma_start(msg[:, :blocks, :],
                                      xs_d[:, c0:c0 + blocks, :])
                elif hf == 0:
                    isl = idxa[:, poff[(gi, hf)]:poff[(gi, hf)] + nidx // 16]
                    nc.gpsimd.dma_gather(msg[:, :blocks, :], tabA, isl,
                                         nidx, nidx, elem_size=C,
                                         single_packet=False,
                                         queue_num=(gi * 2 + hf) % 4)
                else:
                    isl = idxb[:, poff[(gi, hf)]:poff[(gi, hf)] + nidx // 16]
                    nc.gpsimd.dma_gather(msg[:, :blocks, :], tabB, isl,
                                         nidx, nidx, elem_size=C,
                                         single_packet=False,
                                         queue_num=(gi * 2 + hf) % 4)
                msgs[(gi, hf)] = msg

            def epilogues(g0, g1, ps):
                for w in range(g0, g1):
                    aT = epool.tile([128, 128], BF16, tag="aT")
                    nc.scalar.activation(aT[:], ps[w][:], AF.Identity)
                    if is_l1:
                        pre = ppool_e.tile([128, C], F32, tag="pse")
                        nc.tensor.matmul(pre[:], lhsT=aT[:], rhs=w1t[:],
                                         start=True, stop=True)
                        t1 = epool.tile([128, C], F32, tag="t1")
                        nc.vector.tensor_tensor(out=t1[:], in0=pre[:],
                                                in1=b1b[:], op=ALU.add)
                        # table2 row = dinv_d * relu(t1) = relu(dinv_d * t1)
                        g = grp_of[w]
                        nc.scalar.activation(h1st[g][:, w - AGS[g], :], t1[:],
                                             AF.Relu, scale=dinvl[:, w:w + 1])
                        if w == AGS[g] + AGW[g] - 1:
                            nc.sync.dma_start(
                                g2loc[g][:].rearrange("(j p) f -> p j f", p=128),
                                h1st[g][:])
                            nc.gpsimd.collective_compute(
                                "AllGather", ALU.bypass,
                                replica_groups=[list(range(CORES))],
                                ins=[g2loc[g][:]],
                                outs=[g2d[TB[g]:TB[g] + CORES * AGR[g], :]])
                    else:
                        p2 = ppool_e.tile([128, 128], F32, tag="pse")
                        nc.tensor.matmul(p2[:], lhsT=w2t[:], rhs=aT[:],
                                         start=True, stop=True)
                        # dst-side dinv_d (d on the free dim here)
                        t2 = epool.tile([128, 128], F32, tag="t2")
                        nc.vector.tensor_tensor(
                            out=t2[:], in0=p2[:],
                            in1=dinvb[:, w * 128:(w + 1) * 128], op=ALU.mult)
                        h2T = epool.tile([128, 128], BF16, tag="h2T")
                        nc.scalar.activation(h2T[:], t2[:], AF.Relu,
                                             bias=b2c[:, 0:1])
                        yp = ppool_y.tile([128, OUT_C], F32, tag="psy")
                        nc.tensor.matmul(yp[:], lhsT=h2T[:], rhs=wpt[:],
                                         start=True, stop=True)
                        yt = epool.tile([128, OUT_C], F32, tag="yt")
                        nc.vector.tensor_tensor(out=yt[:], in0=yp[:],
                                                in1=bpb[:], op=ALU.add)
                        yr = epool.tile([128, OUT_C], F32, tag="yr")
                        nc.scalar.activation(yr[:], yt[:], AF.Relu)
                        nc.sync.dma_start(out_d[w * 128:(w + 1) * 128, :], yr[:])


            # emit each group's gather pieces just ahead of their matmuls
            for p0 in range(0, len(groups), 1):
                pair = [p0]
                for gj in pair:
                    emit_gather(gj, 0)
                    emit_gather(gj, 1)
                for gj in pair:
                    g0, g1 = groups[gj]
                    gchunks = sum(caps[w][0] + caps[w][1]
                                  for w in range(g0, g1))
                    for cb in range((ci // SELB) * SELB, ci + gchunks, SELB):
                        if cb // SELB not in selts and cb < nchunk:
                            sel_build(cb)
                    # aggregation: aggT[f, d] += msg^T-contraction @ sel
                    done = {w: 0 for w in range(g0, g1)}
                    tot = {w: caps[w][0] + caps[w][1] for w in range(g0, g1)}
                    ps = {}
                    for hf in (0, 1):
                        blk = 0
                        for w in range(g0, g1):
                            for _ in range(caps[w][hf]):
                                sl = sel_slice(ci)
                                ci += 1
                                if w not in ps:
                                    ps[w] = ppool_w.tile([128, 128], F32,
                                                         tag="psw",
                                                         name=f"ps{w}")
                                nc.tensor.matmul(
                                    ps[w][:], lhsT=msgs[(gj, hf)][:, blk, :],
                                    rhs=sl, start=(done[w] == 0),
                                    stop=(done[w] == tot[w] - 1))
                                done[w] += 1
                                blk += 1
                    epilogues(g0, g1, ps)

        h1st = [stgp.tile([128, AGW[g], C], BF16, tag=f"h1st{g}",
                          name=f"h1st{g}") for g in range(len(AGW))]
        layer(None, None, True, h1st)
        # dinvb is only read by the layer-2 epilogue; emitting its
        # DVE reciprocals here keeps them off the startup critical path
        dinvb = cp.tile([128, SP], BF16, tag="dinvb")
        for j in range(0, SP, 1568):
            sqb = epool.tile([128, 1568], F32, tag="sqb")
            nc.scalar.activation(sqb[:], degrow[:, j:j + 1568], AF.Sqrt)
            with nc.allow_low_precision(reason="bf16 dinv, |err|<0.4% ok"):
                nc.vector.reciprocal(dinvb[:, j:j + 1568], sqb[:])

        layer(g2d[0:HALF, :], g2d[HALF:NPAD, :], False, None)

    nc.compile()
    return nc


def _make_in_maps(cfg, shared, per_core, W1, b1, W2, b2, Wp, bp):
    w1t = np.ascontiguousarray(np.asarray(W1, np.float32).T).astype(NP_BF16)
    w2t = np.ascontiguousarray(np.asarray(W2, np.float32).T).astype(NP_BF16)
    wpt = np.ascontiguousarray(np.asarray(Wp, np.float32).T).astype(NP_BF16)
    b1b = np.ascontiguousarray(np.tile(np.asarray(b1, np.float32)[None], (128, 1)))
    b2c = np.ascontiguousarray(np.asarray(b2, np.float32)[:, None])
    bpb = np.ascontiguousarray(np.tile(np.asarray(bp, np.float32)[None], (128, 1)))
    iota = np.ascontiguousarray(np.tile(
        np.repeat(np.arange(128, dtype=np.float32), cfg.SELB)[None],
        (128, 1))).astype(NP_BF16)
    base = dict(w1t=w1t, w2t=w2t, wpt=wpt, b1b=b1b, b2c=b2c,
                bpb=bpb, iota=iota)
    in_maps = []
    for c in range(cfg.CORES):
        pc = per_core[c]
        m = dict(base)
        m["idxa"] = pc["idxa"]
        m["idxb"] = pc["idxb"]
        m["rel"] = pc["rel"]
        m["degs"] = pc["degs"]
        m["degd"] = pc["degd"]
        m["degl"] = pc["degl"]
        m["degrow"] = pc["degrow"]
        m["xs"] = pc["xs"]
        in_maps.append(m)
    return in_maps


def _run(inputs, cfg=None, trace=False, tmpdir=None, verbose=True):
    import time
    t0 = time.time()

    def _log(msg):
        if verbose:
            print(f"[kernel {time.time()-t0:7.1f}s] {msg}", flush=True)
    cfg = cfg or CFG
    sched, shared, per_core = _host_prep(cfg, inputs["x"], inputs["edge_index"])
    _log("host prep done")
    nc = _build_nc(cfg, sched)
    _log("build+compile done")
    in_maps = _make_in_maps(cfg, shared, per_core,
                            inputs["W1"], inputs["b1"], inputs["W2"],
                            inputs["b2"], inputs["Wp"], inputs["bp"])
    _log("in_maps done")
    core_ids = list(range(cfg.CORES))
    if trace:
        # NTFF profiling needs a warm first execute; run once untraced.
        bass_utils.run_bass_kernel_spmd(nc, in_maps, core_ids=core_ids,
                                        trace=False)
        _log("warmup run done")
    res = bass_utils.run_bass_kernel_spmd(nc, in_maps, core_ids=core_ids,
                                          trace=trace, tmpdir=tmpdir)
    _log("run done")
    out = np.empty((cfg.N, cfg.OUT_C), np.float32)
    for c in range(cfg.CORES):
        out[c * cfg.S:(c + 1) * cfg.S] = res.results[c]["out"][:cfg.S]
    return out, res


def kernel(**inputs):
    out, _ = _run(inputs)
    return out
